# revision 20
# baseline (speedup 1.0000x reference)
"""Trainium2 Bass kernel for nn_LocalSmoother (LN -> QKV -> RoPE -> 32-token
block-diagonal attention -> out-proj -> residual).

Sharding: B*L = 16384 tokens split evenly across 8 cores (2048 tokens each,
64 chunks of 32). Attention is block-diagonal over 32-token chunks, so shards
are fully independent (pure SPMD, no collectives). Weights are replicated.

V2 layout strategy (compact block-diagonal attention):
  - LayerNorm in token-partition layout; xn fp16; DMA xbar transpose to
    feature-partition XT.
  - QKV fp16 PE matmuls: q^T/k^T feature-partition, V token-partition.
  - RoPE with flat pre-expanded [128, 512] cos/tan tables; the +-32-partition
    rotate-half shuffle is an SBUF->SBUF DMA; all elementwise on VectorE in
    flat fp16 (2x mode), nothing on GpSimd.
  - Scores: only the 32x32 diagonal blocks, via PE tile packing. Per 128-token
    tile one PSUM bank [128 keys, 16 heads x 32 queries]; MM (head h, chunk j)
    = k_cj.T @ q_cj at tile_position (po, 32j). exp folds scale+key-mask bias.
    No block mask multiply needed -- the layout is exactly block-diagonal.
  - Denominators: one matmul with a [128,4] chunk-indicator lhsT -> [4, 512];
    reciprocal; broadcast back with a [4,128] indicator lhsT -> [128, 512];
    one TT multiply normalizes P.
  - PV: per (head, chunk) 32-contraction MMs at tile_position (32j, po) into a
    [128, 1024] A^T accumulator (feature-partition); out-proj consumes it and
    lands token-partition; residual add + store.

ln_gamma is folded into W_qkv on the host; ln_beta (zero in setup_inputs) is
applied to XT as a per-partition bias pass only when nonzero.
"""

import sys
import numpy as np
from contextlib import ExitStack

sys.path.insert(0, "/opt/trn_rl_repo")

D_MODEL = 1024
N_HEADS = 16
D_HEAD = 64
CHUNK = 32
LN_EPS = 1e-5
ROPE_BASE = 10000.0

N_CORES = 8
BLK = 512          # tokens per pipeline block
SUB = 128          # tokens per partition tile
NSUB = BLK // SUB  # 4
ND = D_MODEL // 128  # 8 feature tiles


def build_program(T, with_beta=False, stop_stage=None, repeats=1):
    """Build the per-core Bass program for a T-token shard."""
    import concourse.bass as bass
    import concourse.tile as tile
    from concourse import bacc, mybir

    dt = mybir.dt
    AF = mybir.ActivationFunctionType
    OP = mybir.AluOpType

    NBLK = T // BLK
    nc = bacc.Bacc("TRN2", target_bir_lowering=False, debug=False,
                   num_devices=N_CORES)

    xs = nc.dram_tensor("xs", [T, D_MODEL], dt.float32, kind="ExternalInput").ap()
    wqk = nc.dram_tensor("wqk", [16, 128, ND, 128], dt.float16, kind="ExternalInput").ap()
    wv = nc.dram_tensor("wv", [2, 128, ND, 512], dt.float16, kind="ExternalInput").ap()
    wo = nc.dram_tensor("wo", [2, 128, ND, 512], dt.float16, kind="ExternalInput").ap()
    cosf = nc.dram_tensor("cosf", [128, BLK], dt.float16, kind="ExternalInput").ap()
    tanf = nc.dram_tensor("tanf", [128, BLK], dt.float16, kind="ExternalInput").ap()
    c4 = nc.dram_tensor("c4", [128, 4], dt.float16, kind="ExternalInput").ap()
    ct4 = nc.dram_tensor("ct4", [4, 128], dt.float16, kind="ExternalInput").ap()
    kb = nc.dram_tensor("kb", [128, T // 128], dt.float32, kind="ExternalInput").ap()
    beta = None
    if with_beta:
        beta = nc.dram_tensor("beta", [128, ND], dt.float32, kind="ExternalInput").ap()
    ys = nc.dram_tensor("ys", [T, D_MODEL], dt.float32, kind="ExternalOutput").ap()

    with tile.TileContext(nc) as tc, ExitStack() as ctx:
        const = ctx.enter_context(tc.tile_pool(name="const", bufs=1))
        # ---- constants ----
        # small tables first (sync queue); big weights as split DMAs on the
        # SWDGE queue in consumption order, so block-0 compute starts after
        # the first slice instead of behind 8MB of weights in one FIFO
        cos_sb = const.tile([128, BLK], dt.float16, tag="cos")
        nc.sync.dma_start(cos_sb[:], cosf)
        tan_sb = const.tile([128, BLK], dt.float16, tag="tan")
        nc.sync.dma_start(tan_sb[:], tanf)
        c4_sb = const.tile([128, 4], dt.float16, tag="c4")
        nc.sync.dma_start(c4_sb[:], c4)
        ct4_sb = const.tile([4, 128], dt.float16, tag="ct4")
        nc.sync.dma_start(ct4_sb[:], ct4)
        kb_sb = const.tile([128, T // 128], dt.float32, tag="kb")
        nc.sync.dma_start(kb_sb[:], kb)
        eps_sb = const.tile([128, 1], dt.float32, tag="eps")
        nc.gpsimd.memset(eps_sb[:], LN_EPS)
        wqk_sb = const.tile([128, 16, ND, 128], dt.float16, tag="wqk")
        for et in range(16):
            nc.gpsimd.dma_start(wqk_sb[:, et], wqk[et])
        wv_sb = const.tile([128, 2, ND, 512], dt.float16, tag="wv")
        for n in range(2):
            nc.gpsimd.dma_start(wv_sb[:, n], wv[n])
        wo_sb = const.tile([128, 2, ND, 512], dt.float16, tag="wo")
        for n in range(2):
            nc.gpsimd.dma_start(wo_sb[:, n], wo[n])
        beta_sb = None
        if with_beta:
            beta_sb = const.tile([128, ND], dt.float32, tag="beta")
            nc.sync.dma_start(beta_sb[:], beta)

        tan_bc = tan_sb[:].unsqueeze(1).to_broadcast((128, ND, BLK))

        # ---- pools ----
        xp = ctx.enter_context(tc.tile_pool(name="xp", bufs=2))
        xnp = ctx.enter_context(tc.tile_pool(name="xnp", bufs=4))
        stp = ctx.enter_context(tc.tile_pool(name="stp", bufs=8))
        scp = ctx.enter_context(tc.tile_pool(name="scp", bufs=1))
        xtp = ctx.enter_context(tc.tile_pool(name="xtp", bufs=2))
        qcp = ctx.enter_context(tc.tile_pool(name="qcp", bufs=2))
        qsp = ctx.enter_context(tc.tile_pool(name="qsp", bufs=2))
        vp = ctx.enter_context(tc.tile_pool(name="vp", bufs=8))
        pep = ctx.enter_context(tc.tile_pool(name="pep", bufs=6))
        rcp = ctx.enter_context(tc.tile_pool(name="rcp", bufs=2))
        pnp = ctx.enter_context(tc.tile_pool(name="pnp", bufs=4))
        asp = ctx.enter_context(tc.tile_pool(name="asp", bufs=4))
        xrp = ctx.enter_context(tc.tile_pool(name="xrp", bufs=2))
        yp = ctx.enter_context(tc.tile_pool(name="yp", bufs=2))

        # PSUM: 8 banks total. psQ 2 (QKV/V projections), psT 2 (scores ->
        # sums -> rcb ring), psB 4 (per-chunk PV accumulators A_j; the two
        # out-proj halves reuse freed A_j slots).
        # HW rule: a PSUM bank must only see ONE contraction row base
        # (tile_position[0]) across concurrent PE writes; column bases may mix.
        psQ = ctx.enter_context(tc.tile_pool(name="psQ", bufs=2, space="PSUM"))
        psT = ctx.enter_context(tc.tile_pool(name="psT", bufs=2, space="PSUM"))
        psB = ctx.enter_context(tc.tile_pool(name="psB", bufs=4, space="PSUM"))

        for b in range(NBLK * repeats):
            b = b % NBLK
            t0 = b * BLK
            # ---------- LayerNorm (token-partition) ----------
            xn_tiles = []
            for tt in range(NSUB):
                xt = xp.tile([128, D_MODEL], dt.float32, tag="x")
                nc.sync.dma_start(xt[:], xs[t0 + tt * SUB: t0 + (tt + 1) * SUB, :])
                scr = scp.tile([128, D_MODEL], dt.float16, tag="scr")
                s1 = stp.tile([128, 1], dt.float32, tag="s1")
                nc.scalar.activation(scr[:], xt[:], AF.Identity, accum_out=s1[:])
                s2 = stp.tile([128, 1], dt.float32, tag="s2")
                nc.scalar.activation(scr[:], xt[:], AF.Square, accum_out=s2[:])
                mu = stp.tile([128, 1], dt.float32, tag="mu")
                nc.vector.tensor_scalar_mul(mu[:], s1[:], 1.0 / D_MODEL)
                mu2 = stp.tile([128, 1], dt.float32, tag="mu2")
                nc.vector.tensor_tensor(mu2[:], mu[:], mu[:], op=OP.mult)
                var = stp.tile([128, 1], dt.float32, tag="var")
                nc.vector.scalar_tensor_tensor(var[:], s2[:], 1.0 / D_MODEL,
                                               mu2[:], op0=OP.mult,
                                               op1=OP.subtract)
                # rstd = exp(-0.5*ln(var+eps)); Ln/Exp/Square/Copy all live in
                # one ACT table set (no table thrash; ACT Rsqrt is banned)
                lv = stp.tile([128, 1], dt.float32, tag="lv")
                nc.scalar.activation(lv[:], var[:], AF.Ln, bias=eps_sb[:])
                rstd = stp.tile([128, 1], dt.float32, tag="rs")
                nc.scalar.activation(rstd[:], lv[:], AF.Exp, scale=-0.5)
                xn = xnp.tile([128, D_MODEL], dt.float16, tag="xn")
                nc.vector.tensor_scalar(xn[:], xt[:], mu[:], rstd[:],
                                        op0=OP.subtract, op1=OP.mult)
                xn_tiles.append(xn)

            if stop_stage == 'ln':
                dbg = yp.tile([128, D_MODEL], dt.float32, tag="y")
                nc.vector.tensor_copy(dbg[:], xn_tiles[0][:])
                nc.sync.dma_start(ys[t0:t0 + SUB, :], dbg[:])
                continue

            # ---------- transpose to feature-partition ----------
            # one 3D-output xbar transpose per token tile: [128, 1024] ->
            # [128, 8 dtiles, 128]. Layout is [128, tt, dtile, 128] so each
            # transpose dest is CONTIGUOUS (the xbar corrupts strided dests);
            # consumers use strided views instead.
            # all transposes on ONE HWDGE queue: concurrent DMA-transposes on
            # both queues corrupt data (shared xbar state)
            XT = xtp.tile([128, NSUB, ND, SUB], dt.float16, tag="xt")
            for tt in range(NSUB):
                nc.scalar.dma_start_transpose(XT[:, tt, :, :], xn_tiles[tt][:])
            if with_beta:
                for dtile in range(ND):
                    nc.scalar.activation(XT[:, :, dtile, :], XT[:, :, dtile, :],
                                         AF.Identity, bias=beta_sb[:, dtile:dtile + 1])

            if stop_stage == 'xt':
                dbg = yp.tile([128, D_MODEL], dt.float32, tag="y")
                nc.vector.tensor_copy(dbg[:, 0:512], XT[:, :, 0, :].rearrange("p a c -> p (a c)"))
                nc.sync.dma_start(ys[t0:t0 + SUB, :], dbg[:])
                continue

            # ---------- qk projection (feature-partition out) + cos fuse ----
            q_all = qcp.tile([128, ND, BLK], dt.float16, tag="qall")
            k_all = qcp.tile([128, ND, BLK], dt.float16, tag="kall")
            for et in range(16):
                ps = psQ.tile([128, BLK], dt.float32, tag="ps")
                for dtile in range(ND):
                    nc.tensor.matmul(ps[:],
                                     wqk_sb[:, et, dtile, :],
                                     XT[:, :, dtile, :],
                                     start=(dtile == 0), stop=(dtile == ND - 1))
                tgt = q_all if et < 8 else k_all
                nc.vector.tensor_tensor(tgt[:, et % 8, :], ps[:], cos_sb[:],
                                        op=OP.mult)

            if stop_stage == 'qk':
                dbg = yp.tile([128, D_MODEL], dt.float32, tag="y")
                nc.vector.tensor_copy(dbg[:, 0:512], q_all[:, 0, :])
                nc.sync.dma_start(ys[t0:t0 + SUB, :], dbg[:])
                continue

            # ---------- v projection (token-partition out) ----------
            v_tiles = []
            for tt in range(NSUB):
                vt = vp.tile([128, D_MODEL], dt.float16, tag="v")
                for n in range(2):
                    ps = psQ.tile([128, BLK], dt.float32, tag="ps")
                    for dtile in range(ND):
                        nc.tensor.matmul(ps[:],
                                         XT[:, tt, dtile, :],
                                         wv_sb[:, n, dtile, :],
                                         start=(dtile == 0), stop=(dtile == ND - 1))
                    nc.scalar.copy(vt[:, n * 512:(n + 1) * 512], ps[:])
                v_tiles.append(vt)

            if stop_stage == 'v':
                dbg = yp.tile([128, D_MODEL], dt.float32, tag="y")
                nc.vector.tensor_copy(dbg[:], v_tiles[0][:])
                nc.sync.dma_start(ys[t0:t0 + SUB, :], dbg[:])
                continue

            # ---------- rope: shuffle (+-32 partitions) and combine ----------
            for src_t, eng in ((q_all, nc.sync), (k_all, nc.sync)):
                qs = qsp.tile([128, ND, BLK], dt.float16, tag="qs")
                for (o, i) in ((0, 32), (32, 0), (64, 96), (96, 64)):
                    eng.dma_start(qs[o:o + 32, :, :], src_t[i:i + 32, :, :])
                nc.vector.tensor_tensor(qs[:], qs[:], tan_bc, op=OP.mult)
                nc.vector.tensor_tensor(src_t[:], src_t[:], qs[:], op=OP.add)

            if stop_stage == 'rope':
                dbg = yp.tile([128, D_MODEL], dt.float32, tag="y")
                nc.vector.tensor_copy(dbg[:, 0:512], q_all[:, 0, :])
                nc.sync.dma_start(ys[t0:t0 + SUB, :], dbg[:])
                continue

            # ---------- attention per 128-token tile ----------
            # Compact block-diagonal scores, parity-split so each PSUM bank
            # sees a single contraction row base: even heads (po=0) in spe,
            # odd heads (po=64) in spo. Free layout of pexp/pn [128, 512]:
            # even head h at 32*(h//2), odd head h at 256 + 32*(h//2).
            for tt in range(NSUB):
                spe = psT.tile([128, 256], dt.float32, tag="att")
                spo = psT.tile([128, 256], dt.float32, tag="att")
                for h in range(N_HEADS):
                    et, po = h // 2, (h % 2) * 64
                    sp = spe if h % 2 == 0 else spo
                    for j in range(4):
                        tok = tt * SUB + 32 * j
                        nc.tensor.matmul(
                            sp[32 * j:32 * j + 32, 32 * et:32 * et + 32],
                            k_all[po:po + 64, et, tok:tok + 32],
                            q_all[po:po + 64, et, tok:tok + 32],
                            start=True, stop=True,
                            tile_position=(po, 32 * j))
                pexp = pep.tile([128, BLK], dt.float16, tag="pe")
                bidx = (t0 // SUB) + tt
                nc.scalar.activation(pexp[:, 0:256], spe[:], AF.Exp,
                                     scale=float(D_HEAD) ** -0.5,
                                     bias=kb_sb[:, bidx:bidx + 1])
                nc.scalar.activation(pexp[:, 256:512], spo[:], AF.Exp,
                                     scale=float(D_HEAD) ** -0.5,
                                     bias=kb_sb[:, bidx:bidx + 1])
                if stop_stage == 'attn1':
                    dbg = yp.tile([128, D_MODEL], dt.float32, tag="y")
                    nc.vector.tensor_copy(dbg[:, 0:512], pexp[:])
                    nc.sync.dma_start(ys[t0 + tt * SUB:t0 + (tt + 1) * SUB, :], dbg[:])
                    continue
                sums = psT.tile([4, BLK], dt.float32, tag="att")
                nc.tensor.matmul(sums[:], c4_sb[:], pexp[:],
                                 start=True, stop=True)
                # rc = 1/sums as exp(-ln(sums)) on ScalarE: ~1.4us vs ~3.2us
                # for the DVE iterative-divide reciprocal, and off VectorE
                ls = rcp.tile([4, BLK], dt.float32, tag="ls")
                nc.scalar.activation(ls[:], sums[:], AF.Ln)
                rc = rcp.tile([4, BLK], dt.float16, tag="rc")
                nc.scalar.activation(rc[:], ls[:], AF.Exp, scale=-1.0)
                rcb = psT.tile([128, BLK], dt.float32, tag="att")
                nc.tensor.matmul(rcb[:], ct4_sb[:], rc[:],
                                 start=True, stop=True)
                pn = pnp.tile([128, BLK], dt.float16, tag="pn")
                nc.vector.tensor_tensor(pn[:], pexp[:], rcb[:], op=OP.mult)
                if stop_stage == 'attn':
                    dbg = yp.tile([128, D_MODEL], dt.float32, tag="y")
                    nc.vector.tensor_copy(dbg[:, 0:512], pn[:])
                    nc.sync.dma_start(ys[t0 + tt * SUB:t0 + (tt + 1) * SUB, :], dbg[:])
                    continue

                # ---------- PV: per-chunk A_j [128 features, 8 et x 32 tok],
                # so bank j only sees contraction row base 32j ----------
                ajs = []
                for j in range(4):
                    aj = psB.tile([128, 256], dt.float32, tag="ab")
                    for h in range(N_HEADS):
                        et, po = h // 2, (h % 2) * 64
                        off = (h % 2) * 256
                        nc.tensor.matmul(
                            aj[po:po + 64, et * 32:et * 32 + 32],
                            v_tiles[tt][32 * j:32 * j + 32, h * D_HEAD:(h + 1) * D_HEAD],
                            pn[32 * j:32 * j + 32, off + 32 * et:off + 32 * et + 32],
                            start=True, stop=True,
                            tile_position=(32 * j, po))
                    ajs.append(aj)
                asb = asp.tile([128, ND, SUB], dt.float16, tag="a")
                for j in range(4):
                    eng = nc.scalar if j % 2 == 0 else nc.vector
                    src = ajs[j][:].rearrange("p (e i) -> p e i", i=32)
                    if j % 2 == 0:
                        eng.copy(asb[:, :, 32 * j:32 * j + 32], src)
                    else:
                        eng.tensor_copy(asb[:, :, 32 * j:32 * j + 32], src)

                if stop_stage == 'pv':
                    dbg = yp.tile([128, D_MODEL], dt.float32, tag="y")
                    nc.vector.tensor_copy(dbg[:], asb[:].rearrange("p a c -> p (a c)"))
                    nc.sync.dma_start(ys[t0 + tt * SUB:t0 + (tt + 1) * SUB, :], dbg[:])
                    continue

                # ---------- out projection + residual ----------
                # two [128, 512] halves; each reuses a freed A_j bank
                oph = []
                for n in range(2):
                    op_ps = psB.tile([128, BLK], dt.float32, tag="ab")
                    for dp in range(ND):
                        nc.tensor.matmul(op_ps[:],
                                         asb[:, dp, :],
                                         wo_sb[:, n, dp, :],
                                         start=(dp == 0), stop=(dp == ND - 1))
                    oph.append(op_ps)
                xr = xrp.tile([128, D_MODEL], dt.float32, tag="xr")
                rows = slice(t0 + tt * SUB, t0 + (tt + 1) * SUB)
                nc.sync.dma_start(xr[:], xs[rows, :])
                y = yp.tile([128, D_MODEL], dt.float32, tag="y")
                for n in range(2):
                    nc.vector.tensor_tensor(y[:, n * 512:(n + 1) * 512], oph[n][:],
                                            xr[:, n * 512:(n + 1) * 512], op=OP.add)
                nc.sync.dma_start(ys[rows, :], y[:])

    nc.compile()
    _unify_act_table_loads(nc)
    return nc


def _unify_act_table_loads(nc):
    """Point every ACT table load at the one set containing all functions we
    use (Ln/Exp/Square/Copy/Identity), then drop now-redundant loads. The
    stock pass maps ln->natural_log and exp->exp_and_others, thrashing the
    table ~60x per program (~1.3us + drain each)."""
    from concourse import mybir
    from concourse.hw_specs import get_activation_tables

    AF = mybir.ActivationFunctionType
    need = {AF.Ln, AF.Exp, AF.Square, AF.Copy, AF.Identity}
    tables = get_activation_tables(nc.m.arch)
    set_id = None
    for idx, (name, funcs) in enumerate(tables.items()):
        if need <= funcs:
            set_id = idx
            break
    assert set_id is not None, "no ACT table set covers Ln+Exp+Square+Copy"
    for blk in nc.m.functions[0].blocks:
        seen = False
        keep = []
        for inst in blk.instructions:
            if isinstance(inst, mybir.InstLoadActFuncSet):
                if seen:
                    continue            # redundant once unified; drop
                inst.act_func_set_id = set_id
                seen = True
            keep.append(inst)
        blk.instructions[:] = keep


def host_inputs(x, mask, ln_gamma, ln_beta, W_qkv, W_out, T):
    """Prepare per-core input maps. x: (B, L, D) fp32."""
    B, L, D = x.shape
    tokens = B * L
    n_cores = tokens // T
    W_eff = (W_qkv * ln_gamma[None, :]).astype(np.float32)
    wqk_h = W_eff[0:2 * D].T.astype(np.float16)      # (D, 2D) d-major
    wv_h = W_eff[2 * D:3 * D].T.astype(np.float16)   # (D, D)
    wo_h = W_out.T.astype(np.float16)
    # consumption-ordered layouts: wqk (16 et, 128 p, 8 dt, 128); wv/wo (2 n, ...)
    wqk_h = np.ascontiguousarray(
        wqk_h.reshape(ND, 128, 16, 128).transpose(2, 1, 0, 3))
    wv_h = np.ascontiguousarray(
        wv_h.reshape(ND, 128, 2, 512).transpose(2, 1, 0, 3))
    wo_h = np.ascontiguousarray(
        wo_h.reshape(ND, 128, 2, 512).transpose(2, 1, 0, 3))

    inv_freq = 1.0 / (ROPE_BASE ** (np.arange(0, D_HEAD, 2) / D_HEAD))  # (32,)
    p = np.arange(128)
    j = p % D_HEAD
    idx = j % 32
    sign = np.where(j < 32, -1.0, 1.0)
    t = np.arange(CHUNK)
    ang = t[None, :] * inv_freq[idx][:, None]          # (128, 32)
    cos_h = np.cos(ang).astype(np.float16)
    tan_h = (sign[:, None] * np.tan(ang)).astype(np.float16)
    cos_flat = np.tile(cos_h, (1, BLK // CHUNK)).astype(np.float16)   # (128, 512)
    tan_flat = np.tile(tan_h, (1, BLK // CHUNK)).astype(np.float16)

    pp = np.arange(128)
    c4_h = (pp[:, None] // CHUNK == np.arange(4)[None, :]).astype(np.float16)
    ct4_h = np.ascontiguousarray(c4_h.T)

    xs_flat = np.ascontiguousarray(x.reshape(tokens, D).astype(np.float32))
    mask_flat = mask.reshape(tokens).astype(np.float32)
    kbias = np.where(mask_flat == 0, -30000.0, 0.0).astype(np.float32)

    shared = {"wqk": wqk_h, "wv": wv_h, "wo": wo_h,
              "cosf": cos_flat, "tanf": tan_flat, "c4": c4_h, "ct4": ct4_h}
    with_beta = bool(np.any(ln_beta != 0))
    if with_beta:
        shared["beta"] = np.ascontiguousarray(
            ln_beta.reshape(ND, 128).T).astype(np.float32)

    in_maps = []
    for c in range(n_cores):
        sl = slice(c * T, (c + 1) * T)
        kb_c = np.ascontiguousarray(
            kbias[sl].reshape(T // 128, 128).T).astype(np.float32)
        m = dict(shared)
        m["xs"] = xs_flat[sl]
        m["kb"] = kb_c
        in_maps.append(m)
    return in_maps, with_beta


_PROGRAM_CACHE = {}


def kernel(x, mask, ln_gamma, ln_beta, W_qkv, W_out):
    from concourse import bass_utils

    x = np.asarray(x, dtype=np.float32)
    mask = np.asarray(mask, dtype=np.float32)
    ln_gamma = np.asarray(ln_gamma, dtype=np.float32)
    ln_beta = np.asarray(ln_beta, dtype=np.float32)
    W_qkv = np.asarray(W_qkv, dtype=np.float32)
    W_out = np.asarray(W_out, dtype=np.float32)

    B, L, D = x.shape
    T = (B * L) // N_CORES
    in_maps, with_beta = host_inputs(x, mask, ln_gamma, ln_beta, W_qkv, W_out, T)

    key = (T, with_beta)
    if key not in _PROGRAM_CACHE:
        _PROGRAM_CACHE[key] = build_program(T, with_beta=with_beta)
    nc = _PROGRAM_CACHE[key]

    res = bass_utils.run_bass_kernel_spmd(nc, in_maps, core_ids=list(range(N_CORES)))
    ys = np.concatenate([res.results[c]["ys"] for c in range(N_CORES)], axis=0)
    return ys.reshape(B, L, D).astype(np.float32)


if __name__ == "__main__":
    rng = np.random.default_rng(0)
    B, L = 4, 4096
    x = rng.standard_normal((B, L, D_MODEL), dtype=np.float32)
    mask = np.ones((B, L), dtype=np.float32)
    g = np.ones(D_MODEL, dtype=np.float32)
    be = np.zeros(D_MODEL, dtype=np.float32)
    Wq = (rng.standard_normal((3 * D_MODEL, D_MODEL)) * 0.02).astype(np.float32)
    Wo = (rng.standard_normal((D_MODEL, D_MODEL)) * 0.02).astype(np.float32)
    y = kernel(x, mask, g, be, Wq, Wo)
    print("kernel output:", y.shape, y.dtype)
